# revision 13
# baseline (speedup 1.0000x reference)
"""Multi-head attention (B=2, S=4096, D=768, H=12) on 8 Trainium2 cores.

Sharding: batch x heads. Core c = (b, r) with b = c // 4, r = c % 4 handles
batch b and heads {3r, 3r+1, 3r+2}. Each core:
  phase 0: load + fp32r-round weights, build identity.
  phase 1: transpose x -> x^T (PE), project Q^T/K^T (head-dim on partitions)
           and V' = [V | 1] (natural layout, ones column for softmax sums).
  phase 2: per (head, 512-query supertile): S^T = K @ Q^T chunks -> exp (ACT,
           scale=1/8, no max subtraction: |scores/8| < ~6) -> P^T, then
           out'^T = V'^T-accumulate over all 32 key chunks (row 64 = softmax
           denominators). Normalize via reciprocal + PE broadcast.
  phase 3: out-projection partials for its 3 heads (+ bo/4), staged
           ReduceScatter(add) over the 4 cores of the batch -> each core owns
           a quarter of the rows.
Host: slices weights per core, reassembles row quarters. All heavy matmuls run
in fp32r (1 cycle/row at N>=256).
"""
import numpy as np

B, S, D, NH, HD = 2, 4096, 768, 12, 64
P = 128
NCORES = 8
SCALE = 0.125  # 1/sqrt(64)

_CACHE = {}


def _build(bench_reps=1, loop_reps=None, no_rs=False):
    import concourse.bacc as bacc
    import concourse.mybir as mybir
    import concourse.tile as tile
    from concourse.masks import make_identity

    F32 = mybir.dt.float32
    F32R = mybir.dt.float32r
    AF = mybir.ActivationFunctionType

    nc = bacc.Bacc("TRN2", target_bir_lowering=False, debug=False,
                   num_devices=NCORES)

    x_d = nc.dram_tensor("x", [S, D], F32, kind="ExternalInput")
    wqk_d = nc.dram_tensor("wqk", [D, 384], F32, kind="ExternalInput")
    bqk_d = nc.dram_tensor("bqk", [P, 4], F32, kind="ExternalInput")
    wv_d = nc.dram_tensor("wv", [D, 256], F32, kind="ExternalInput")
    bvp_d = nc.dram_tensor("bvp", [1, 256], F32, kind="ExternalInput")
    wo_d = nc.dram_tensor("wo", [192, D], F32, kind="ExternalInput")
    bo4_d = nc.dram_tensor("bo4", [1, D], F32, kind="ExternalInput")
    y_d = nc.dram_tensor("y", [4, 256, D], F32, kind="ExternalOutput")

    with tile.TileContext(nc) as tc:
        cst = tc.alloc_tile_pool(name="cst", bufs=1)
        per = tc.alloc_tile_pool(name="per", bufs=1)
        dram = tc.alloc_tile_pool(name="dram", bufs=1, space="DRAM")

        ident = cst.tile([P, P], F32)
        make_identity(nc, ident)
        ones64 = cst.tile([1, 64], F32)
        nc.vector.memset(ones64[:], 1.0)
        bqk_s = cst.tile([P, 4], F32)
        nc.sync.dma_start(bqk_s[:], bqk_d[:])

        wqk_s = cst.tile([P, 6, 384], F32R)
        wv_s = cst.tile([P, 6, 256], F32R)
        wo_s = cst.tile([64, 3, D], F32R)
        bvp_r = cst.tile([1, 256], F32R)
        bo4_r = cst.tile([1, D], F32R)
        ones1r = cst.tile([1, P], F32R)

        with tc.tile_pool(name="wtmp", bufs=1) as wtmp:
            t = wtmp.tile([P, 6, 384], F32)
            nc.sync.dma_start(t[:], wqk_d.rearrange("(o p) m -> p o m", p=P))
            nc.vector.tensor_copy(wqk_s[:], t[:])
            t = wtmp.tile([P, 6, 256], F32)
            nc.sync.dma_start(t[:], wv_d.rearrange("(o p) m -> p o m", p=P))
            nc.vector.tensor_copy(wv_s[:], t[:])
            t = wtmp.tile([64, 3, D], F32)
            nc.sync.dma_start(t[:], wo_d.rearrange("(j p) n -> p j n", p=64))
            nc.vector.tensor_copy(wo_s[:], t[:])
            t = wtmp.tile([1, 256], F32)
            nc.sync.dma_start(t[:], bvp_d[:])
            nc.vector.tensor_copy(bvp_r[:], t[:])
            t = wtmp.tile([1, D], F32)
            nc.sync.dma_start(t[:], bo4_d[:])
            nc.vector.tensor_copy(bo4_r[:], t[:])
            t = wtmp.tile([1, P], F32)
            nc.vector.memset(t[:], 1.0)
            nc.vector.tensor_copy(ones1r[:], t[:])

        # persistent per-core tensors
        tQ01 = per.tile([P, S], F32R)   # Q^T head0 rows 0:64, head1 rows 64:128
        tK01 = per.tile([P, S], F32R)
        tQ2 = per.tile([64, S], F32R)
        tK2 = per.tile([64, S], F32R)
        vp = per.tile([P, 32, 195], F32R)  # V' chunks: [V_h | 1] at 65*j

        # ---- phases (repeated bench_reps times for timing) ----
        if loop_reps is None:
            for _rep in range(bench_reps):
                ct = _phases(nc, tc, tile, mybir, AF, F32, F32R,
                             x_d, y_d, dram, ident, ones64, bqk_s, wqk_s,
                             wv_s, wo_s, bvp_r, bo4_r, ones1r, tQ01, tK01,
                             tQ2, tK2, vp, rs_inline=not no_rs)
                if no_rs:
                    for qg in range(4):
                        nc.sync.dma_start(y_d[qg], ct[qg][0:256, :])
        else:
            # hardware For_i loop around the body; collectives hoisted out
            with tc.For_i(0, loop_reps, 1):
                cc_tiles = _phases(nc, tc, tile, mybir, AF, F32, F32R,
                                   x_d, y_d, dram, ident, ones64, bqk_s,
                                   wqk_s, wv_s, wo_s, bvp_r, bo4_r, ones1r,
                                   tQ01, tK01, tQ2, tK2, vp, rs_inline=False)
            for qg in range(4):
                cc_out = dram.tile([256, D], F32, tag="ccout", bufs=2)
                nc.gpsimd.collective_compute(
                    "ReduceScatter", mybir.AluOpType.add,
                    replica_groups=[[0, 1, 2, 3], [4, 5, 6, 7]],
                    ins=[cc_tiles[qg].opt()], outs=[cc_out.opt()])
                nc.sync.dma_start(y_d[qg], cc_out[:])

        dram.release()
        per.release()
        cst.release()

    nc.compile()
    return nc


def _build_loop(loop_reps):
    return _build(loop_reps=loop_reps)


def _phases(nc, tc, tile, mybir, AF, F32, F32R,
            x_d, y_d, dram, ident, ones64, bqk_s, wqk_s, wv_s, wo_s,
            bvp_r, bo4_r, ones1r, tQ01, tK01, tQ2, tK2, vp,
            rs_inline=True):
        cc_tiles = []
        P = 128
        S, D = 4096, 768
        SCALE = 0.125
        # ---- phase 1: x^T, projections ----
        with (
            tc.tile_pool(name="p1", bufs=2) as p1,
            tc.tile_pool(name="p1ps", bufs=1, space="PSUM") as p1ps,
        ):
            # (dst, col base in wqk_s, M, bias col)
            groups = [(tQ01, 0, P, 0), (tK01, P, P, 1),
                      (tQ2, 256, 64, 2), (tK2, 320, 64, 3)]
            for sc in range(8):
                x_nat = p1.tile([P, 4, D], F32, tag="xnat")
                nc.sync.dma_start(
                    x_nat[:],
                    x_d[512 * sc:512 * (sc + 1), :].rearrange(
                        "(j p) d -> p j d", p=P))
                xts = p1.tile([P, 6, 512], F32R, tag="xts")
                for f in range(6):
                    tp = p1ps.tile([P, 512], F32, tag="tp", bufs=2)
                    for j in range(4):
                        nc.tensor.transpose(
                            tp[:, P * j:P * (j + 1)],
                            x_nat[:, j, P * f:P * (f + 1)], ident[:])
                    nc.scalar.copy(xts[:, f, :], tp[:])
                for dst, cb, M, bcol in groups:
                    qk_ps = p1ps.tile([P, 512], F32, tag="qkps", bufs=2)
                    for f in range(6):
                        nc.tensor.matmul(qk_ps[:M, :], wqk_s[:, f, cb:cb + M],
                                         xts[:, f, :],
                                         start=(f == 0), stop=(f == 5))
                    nc.scalar.activation(dst[0:M, 512 * sc:512 * (sc + 1)],
                                         qk_ps[:M, :], AF.Identity,
                                         bias=bqk_s[0:M, bcol:bcol + 1],
                                         scale=1.0)
                for j in range(4):
                    v_ps = p1ps.tile([P, 256], F32, tag="vps", bufs=2)
                    for f in range(6):
                        nc.tensor.matmul(v_ps[:], xts[:, f, P * j:P * (j + 1)],
                                         wv_s[:, f, :],
                                         start=(f == 0), stop=False)
                    nc.tensor.matmul(v_ps[:], ones1r[:], bvp_r[:],
                                     start=False, stop=True)
                    nc.scalar.copy(vp[:, 4 * sc + j, :], v_ps[:, 0:195])

        # ---- phase 2+3: attention, out-projection, reduce-scatter ----
        head_cfg = [(tQ01, tK01, 0), (tQ01, tK01, 64), (tQ2, tK2, 0)]
        with (
            tc.tile_pool(name="p2", bufs=1) as p2,
            tc.tile_pool(name="p2ps", bufs=1, space="PSUM") as p2ps,
        ):
            for qg in range(4):
                cc_in = dram.tile([1024, D], F32, tag="ccin", bufs=2)
                for q2 in range(2):
                    qs = 2 * qg + q2
                    qoff = 512 * qs
                    outTs = []
                    for j in range(3):
                        qt, kt, base = head_cfg[j]
                        o_ps = p2ps.tile([65, 512], F32, tag="ops")
                        for g in range(16):
                            s_ps = p2ps.tile([P, 1024], F32, tag="sps",
                                             bufs=2)
                            for t in range(2):
                                kc = 2 * g + t
                                nc.tensor.matmul(
                                    s_ps[:, 512 * t:512 * (t + 1)],
                                    kt[base:base + 64, P * kc:P * (kc + 1)],
                                    qt[base:base + 64, qoff:qoff + 512],
                                    start=True, stop=True)
                            pt = p2.tile([P, 1024], F32R, tag="pt", bufs=4)
                            nc.scalar.activation(pt[:], s_ps[:], AF.Exp,
                                                 scale=SCALE)
                            for t in range(2):
                                kc = 2 * g + t
                                nc.tensor.matmul(
                                    o_ps[:], vp[:, kc, 65 * j:65 * (j + 1)],
                                    pt[:, 512 * t:512 * (t + 1)],
                                    start=(kc == 0), stop=(kc == 31))
                        srow = p2.tile([1, 512], F32, tag="srow", bufs=2)
                        nc.vector.tensor_copy(srow[:], o_ps[64:65, :])
                        recip = p2.tile([1, 512], F32, tag="recip", bufs=2)
                        nc.vector.reciprocal_approx_fast(recip[:], srow[:])
                        b_ps = p2ps.tile([64, 512], F32, tag="bps")
                        nc.tensor.matmul(b_ps[:], ones64[:], recip[:],
                                         start=True, stop=True)
                        bc = p2.tile([64, 512], F32, tag="bc", bufs=2)
                        nc.vector.tensor_copy(bc[:], b_ps[:])
                        oT = p2.tile([64, 512], F32R, tag="outT", bufs=6)
                        nc.vector.tensor_tensor(oT[:], o_ps[0:64, :], bc[:],
                                                mybir.AluOpType.mult)
                        outTs.append(oT)
                    for t in range(4):
                        f_ps = p2ps.tile([P, D], F32, tag="fps")
                        for j in range(3):
                            nc.tensor.matmul(
                                f_ps[:, 0:512],
                                outTs[j][:, P * t:P * (t + 1)],
                                wo_s[:, j, 0:512],
                                start=(j == 0), stop=False)
                            nc.tensor.matmul(
                                f_ps[:, 512:D],
                                outTs[j][:, P * t:P * (t + 1)],
                                wo_s[:, j, 512:D],
                                start=(j == 0), stop=False)
                        nc.tensor.matmul(f_ps[:, 0:512], ones1r[:],
                                         bo4_r[0:1, 0:512],
                                         start=False, stop=True)
                        nc.tensor.matmul(f_ps[:, 512:D], ones1r[:],
                                         bo4_r[0:1, 512:D],
                                         start=False, stop=True)
                        fout = p2.tile([P, D], F32, tag="fout", bufs=3)
                        nc.vector.tensor_copy(fout[:], f_ps[:])
                        nc.sync.dma_start(
                            cc_in[512 * q2 + P * t:512 * q2 + P * (t + 1), :],
                            fout[:])
                if rs_inline:
                    cc_out = dram.tile([256, D], F32, tag="ccout", bufs=2)
                    nc.gpsimd.collective_compute(
                        "ReduceScatter", mybir.AluOpType.add,
                        replica_groups=[[0, 1, 2, 3], [4, 5, 6, 7]],
                        ins=[cc_in.opt()], outs=[cc_out.opt()])
                    nc.sync.dma_start(y_d[qg], cc_out[:])
                else:
                    cc_tiles.append(cc_in)
        return cc_tiles  # noqa: B012


def _get_nc(bench_reps=1):
    key = ("nc", bench_reps)
    if key not in _CACHE:
        if isinstance(bench_reps, tuple) and bench_reps[0] == "loop":
            _CACHE[key] = _build_loop(bench_reps[1])
        else:
            _CACHE[key] = _build(bench_reps)
    return _CACHE[key]


def _make_in_maps(x, Wq, bq, Wk, bk, Wv, bv, Wo, bo):
    in_maps = []
    for c in range(NCORES):
        b, r = divmod(c, 4)
        hs = [3 * r, 3 * r + 1, 3 * r + 2]
        col = lambda W, h: W[:, HD * h:HD * (h + 1)]
        seg = lambda v, h: v[HD * h:HD * (h + 1)]

        wqk = np.zeros((D, 384), np.float32)
        wqk[:, 0:64] = col(Wq, hs[0]); wqk[:, 64:128] = col(Wq, hs[1])
        wqk[:, 128:192] = col(Wk, hs[0]); wqk[:, 192:256] = col(Wk, hs[1])
        wqk[:, 256:320] = col(Wq, hs[2]); wqk[:, 320:384] = col(Wk, hs[2])

        bqk = np.zeros((P, 4), np.float32)
        bqk[0:64, 0] = seg(bq, hs[0]); bqk[64:128, 0] = seg(bq, hs[1])
        bqk[0:64, 1] = seg(bk, hs[0]); bqk[64:128, 1] = seg(bk, hs[1])
        bqk[0:64, 2] = seg(bq, hs[2]); bqk[0:64, 3] = seg(bk, hs[2])

        wv = np.zeros((D, 256), np.float32)
        bvp = np.zeros((1, 256), np.float32)
        for j in range(3):
            wv[:, 65 * j:65 * j + 64] = col(Wv, hs[j])
            bvp[0, 65 * j:65 * j + 64] = seg(bv, hs[j])
            bvp[0, 65 * j + 64] = 1.0

        wo = np.concatenate([Wo[HD * h:HD * (h + 1), :] for h in hs], axis=0)
        in_maps.append({
            "x": np.ascontiguousarray(x[b]),
            "wqk": wqk, "bqk": bqk, "wv": wv, "bvp": bvp,
            "wo": np.ascontiguousarray(wo.astype(np.float32)),
            "bo4": (bo * 0.25).astype(np.float32).reshape(1, D),
        })
    return in_maps


def _assemble(results):
    out = np.zeros((B, S, D), np.float32)
    for c in range(NCORES):
        b, r = divmod(c, 4)
        y = results[c]["y"]
        for g in range(4):
            out[b, 1024 * g + 256 * r:1024 * g + 256 * (r + 1), :] = y[g]
    return out


def kernel(x, Wq, bq, Wk, bk, Wv, bv, Wo, bo):
    from concourse.bass_utils import run_bass_kernel_spmd
    args = [np.asarray(a, np.float32) for a in
            (x, Wq, bq, Wk, bk, Wv, bv, Wo, bo)]
    nc = _get_nc()
    in_maps = _make_in_maps(*args)
    res = run_bass_kernel_spmd(nc, in_maps, core_ids=list(range(NCORES)))
    return _assemble(res.results)


# ---------------------------------------------------------------------------
# Timing support (used by test.py, not by the grading path): runs the NEFF
# `n` times back-to-back inside one jit dispatch by feeding each run's outputs
# forward as the next run's donated-output operands. The marginal time per
# chained run is the hardware execution time without host dispatch overhead.
def _runner(bench_reps=1):
    import jax
    import numpy as _np
    import concourse.mybir as mybir
    from jax.sharding import Mesh, PartitionSpec
    from jax.experimental.shard_map import shard_map
    from concourse.bass2jax import (_bass_exec_p, install_neuronx_cc_hook,
                                    partition_id_tensor)

    install_neuronx_cc_hook()
    nc = _get_nc(bench_reps)

    partition_name = (nc.partition_id_tensor.name
                      if nc.partition_id_tensor else None)
    in_names, out_names, out_avals = [], [], []
    for alloc in nc.m.functions[0].allocations:
        if not isinstance(alloc, mybir.MemoryLocationSet):
            continue
        name = alloc.memorylocations[0].name
        if alloc.kind == "ExternalInput":
            if name != partition_name:
                in_names.append(name)
        elif alloc.kind == "ExternalOutput":
            out_names.append(name)
            out_avals.append(jax.core.ShapedArray(
                tuple(alloc.tensor_shape), mybir.dt.np(alloc.dtype)))
    n_params = len(in_names)
    all_names = in_names + out_names
    if partition_name is not None:
        all_names.append(partition_name)

    def _body(*args):
        ins = list(args[:n_params])
        outs = list(args[n_params:])
        extra = ([partition_id_tensor()] if partition_name is not None else [])
        outs = list(_bass_exec_p.bind(
            *ins, *outs, *extra,
            out_avals=tuple(out_avals),
            in_names=tuple(all_names),
            out_names=tuple(out_names),
            lowering_input_output_aliases=(),
            sim_require_finite=True,
            sim_require_nnan=True,
            nc=nc,
        ))
        return tuple(outs)

    devices = jax.devices()[:NCORES]
    mesh = Mesh(_np.asarray(devices), ("core",))
    nio = n_params + len(out_names)
    fn = jax.jit(
        shard_map(_body, mesh=mesh,
                  in_specs=(PartitionSpec("core"),) * nio,
                  out_specs=(PartitionSpec("core"),) * len(out_names),
                  check_rep=False),
        donate_argnums=tuple(range(n_params, nio)),
        keep_unused=True,
    )
    return fn, in_names, out_names, out_avals


def measure_exec_ns(inputs, n_chain=3, reps=3):
    """Return (per-exec-ns, results, t_1x_ns, t_Nx_ns). n_chain = in-NEFF
    repetition count of the whole workload; marginal time is the HW time."""
    import time
    import jax
    args = [np.asarray(inputs[k], np.float32) for k in
            ("x", "Wq", "bq", "Wk", "bk", "Wv", "bv", "Wo", "bo")]
    in_maps = _make_in_maps(*args)

    def prep(n):
        fn, in_names, out_names, out_avals = _runner(n)
        concat_in = [np.concatenate([m[k] for m in in_maps], axis=0)
                     for k in in_names]
        dev_in = [jax.device_put(a) for a in concat_in]

        def fresh_zeros():
            return [jax.device_put(
                np.zeros((NCORES * a.shape[0], *a.shape[1:]), a.dtype))
                for a in out_avals]
        return fn, dev_in, fresh_zeros, out_names, out_avals

    def timeit(n):
        fn, dev_in, fresh_zeros, out_names, out_avals = prep(n)
        out = fn(*dev_in, *fresh_zeros())
        jax.block_until_ready(out)
        ts = []
        for _ in range(reps):
            z = fresh_zeros()
            jax.block_until_ready(z)
            t0 = time.perf_counter()
            out = fn(*dev_in, *z)
            jax.block_until_ready(out)
            ts.append(time.perf_counter() - t0)
        return min(ts), out, out_names, out_avals

    t1, out, out_names, out_avals = timeit(1)
    tN, _, _, _ = timeit(n_chain)

    per_exec_ns = (tN - t1) / (n_chain - 1) * 1e9
    results = [
        {name: np.asarray(out[i]).reshape(NCORES, *out_avals[i].shape)[c]
         for i, name in enumerate(out_names)}
        for c in range(NCORES)
    ]
    return per_exec_ns, _assemble(results), t1 * 1e9, tN * 1e9


def _bench_main(bench_reps, reps):
    """Subprocess entry: time one NEFF (workload repeated bench_reps times
    on-device) and print min wall ns as JSON."""
    import json
    import time
    import jax
    rng = np.random.default_rng(0)
    ins = {
        "x": rng.standard_normal((B, S, D)).astype(np.float32),
        "Wq": rng.standard_normal((D, D)).astype(np.float32) * 0.036,
        "bq": rng.standard_normal((D,)).astype(np.float32) * 0.036,
        "Wk": rng.standard_normal((D, D)).astype(np.float32) * 0.036,
        "bk": rng.standard_normal((D,)).astype(np.float32) * 0.036,
        "Wv": rng.standard_normal((D, D)).astype(np.float32) * 0.036,
        "bv": rng.standard_normal((D,)).astype(np.float32) * 0.036,
        "Wo": rng.standard_normal((D, D)).astype(np.float32) * 0.036,
        "bo": rng.standard_normal((D,)).astype(np.float32) * 0.036,
    }
    args = [ins[k] for k in ("x", "Wq", "bq", "Wk", "bk", "Wv", "bv",
                             "Wo", "bo")]
    in_maps = _make_in_maps(*args)
    fn, in_names, out_names, out_avals = _runner(bench_reps)
    concat_in = [np.concatenate([m[k] for m in in_maps], axis=0)
                 for k in in_names]
    dev_in = [jax.device_put(a) for a in concat_in]

    def fresh_zeros():
        return [jax.device_put(
            np.zeros((NCORES * a.shape[0], *a.shape[1:]), a.dtype))
            for a in out_avals]

    out = fn(*dev_in, *fresh_zeros())
    jax.block_until_ready(out)
    ts = []
    for _ in range(reps):
        z = fresh_zeros()
        jax.block_until_ready(z)
        t0 = time.perf_counter()
        out = fn(*dev_in, *z)
        jax.block_until_ready(out)
        ts.append(time.perf_counter() - t0)
    print(json.dumps({"bench_reps": bench_reps,
                      "wall_ns": [t * 1e9 for t in ts],
                      "min_wall_ns": min(ts) * 1e9}))


if __name__ == "__main__":
    import sys
    if "--loop" in sys.argv:
        br = ("loop", int(sys.argv[sys.argv.index("--loop") + 1]))
    elif "--bench" in sys.argv:
        br = int(sys.argv[sys.argv.index("--bench") + 1])
    else:
        br = 1
    rp = int(sys.argv[sys.argv.index("--reps") + 1]) if "--reps" in sys.argv else 4
    _bench_main(br, rp)
